# revision 3
# baseline (speedup 1.0000x reference)
"""Trainium2 Bass kernel for nn_DAtt_ZS (TCN + channel attention + CAM gate).

Self-contained: host-side folding/sharding + Bass/Tile device program.
Data-parallel over batch: 16 batch elements -> 8 cores x 2 (batch-packed
on the 128 SBUF partitions via block-diagonal weights).

Host-side algebraic folding applied (all exact):
  - x transposed to [C, T] channel-major layout
  - eval-mode BatchNorm folded into the adjacent 1x1 convs
  - the pe[:B] quirk (a per-batch-element [64] vector, constant over T)
    folded into the Q/K/V1/V2 bias vectors
  - 3-layer linear head collapsed into one [1, 64] matmul (+ bias)
  - all 0.5 scalings folded into the softmax scores / head bias:
      o1' = 0.5*o1, o2' = 0.5*o2 (via 0.5*softmax)
      cam_in = o1' + o2';  taff-head:  out = sigmoid(w_eff.(gate*(o1'-o2')) + b')
      with b' = b_eff + 0.5*sum(w_eff)
"""

import math
import numpy as np

EPS = 1e-5
BN_SCALE = 1.0 / math.sqrt(1.0 + EPS)

B_FULL = 16
N_CORES = 8
B_PER = B_FULL // N_CORES  # 2
T_FULL = 8192
F_IN = 10
D = 64
BLK = 512     # time block for streaming matmuls
TB = 128      # time block for the qT/kT score accumulation

F32 = None  # set after concourse import (lazy so host-side numpy folding is importable anywhere)

# ---------------------------------------------------------------------------
# weight blob layout (host and device share this)
# ---------------------------------------------------------------------------

CH = [(F_IN, 16), (16, 32), (32, 64)]  # TCN level channels


def _wblob_layout():
    """Returns (slices, total_cols): name -> (col_off, M, K)."""
    slices = {}
    col = 0

    def add(name, k, m):
        nonlocal col
        slices[name] = (col, m, k)
        col += m

    for lvl, (cin, cout) in enumerate(CH):
        k1, k2 = 2 * cin, 2 * cout
        for q in range(3):
            add(f"w1_{lvl}_{q}", k1, k2)
        for q in range(3):
            add(f"w2_{lvl}_{q}", k2, k2)
        add(f"wd_{lvl}", k1, k2)
    add("qk", 128, 256)       # moving rhs for qT/kT: [q_b0|q_b1|k_b0|k_b1]
    add("v1", 128, 128)
    add("v2", 128, 128)
    add("c3", 128, 32)
    add("c4", 32, 128)
    add("c1", 128, 32)
    add("c2", 32, 128)
    add("head", 128, 2)
    add("id64", 64, 64)
    total = ((col + 31) // 32) * 32
    return slices, total


# bias blob columns ([128, NB])
BIAS_COLS = {
    "b1_0": 0, "bs_0": 1, "b1_1": 2, "bs_1": 3, "b1_2": 4, "bs_2": 5,
    "bv1": 6, "bv2": 7, "b3f": 8, "b1f": 9, "gconst": 10, "beff": 11,
}
NB = 16


# ---------------------------------------------------------------------------
# host-side folding
# ---------------------------------------------------------------------------

def _np(a):
    return np.asarray(a, dtype=np.float32)


def _blkdiag(a, b):
    k, m = a.shape
    out = np.zeros((2 * k, 2 * m), np.float32)
    out[:k, :m] = a
    out[k:, m:] = b
    return out


def _fold_params(params, pe):
    """Returns (wblob [128, NW] shared, per_core_biases list of ([128,NB], qkb [128,256]))."""
    slices, total = _wblob_layout()
    wblob = np.zeros((128, total), np.float32)

    def put(name, mat):
        c0, m, k = slices[name]
        assert mat.shape == (k, m), (name, mat.shape, (k, m))
        wblob[:k, c0:c0 + m] = mat

    tcn = params["tcn"]
    for lvl, (cin, cout) in enumerate(CH):
        p = tcn[lvl]
        w1, w2, wd = _np(p["w1"]), _np(p["w2"]), _np(p["wd"])
        for q in range(3):
            put(f"w1_{lvl}_{q}", _blkdiag(w1[:, :, q].T, w1[:, :, q].T))
            put(f"w2_{lvl}_{q}", _blkdiag(w2[:, :, q].T, w2[:, :, q].T))
        put(f"wd_{lvl}", _blkdiag(wd.T, wd.T))

    wq, wk = _np(params["wq"]), _np(params["wk"])
    wv1, wv2 = _np(params["wv1"]), _np(params["wv2"])
    qkmat = np.zeros((128, 256), np.float32)
    qkmat[0:64, 0:64] = wq.T
    qkmat[64:128, 64:128] = wq.T
    qkmat[0:64, 128:192] = wk.T
    qkmat[64:128, 192:256] = wk.T
    put("qk", qkmat)
    put("v1", _blkdiag(wv1.T, wv1.T))
    put("v2", _blkdiag(wv2.T, wv2.T))

    cam = params["cam"]

    def fold_bn(w, b, g, be):
        s = _np(g) * BN_SCALE
        return _np(w) * s[:, None], _np(b) * s + _np(be)

    c1f, b1f = fold_bn(cam["c1w"], cam["c1b"], cam["g1"], cam["be1"])
    c2f, b2f = fold_bn(cam["c2w"], cam["c2b"], cam["g2"], cam["be2"])
    c3f, b3f = fold_bn(cam["c3w"], cam["c3b"], cam["g3"], cam["be3"])
    c4f, b4f = fold_bn(cam["c4w"], cam["c4b"], cam["g4"], cam["be4"])
    put("c3", _blkdiag(c3f.T, c3f.T))
    put("c4", _blkdiag(c4f.T, c4f.T))
    put("c1", _blkdiag(c1f.T, c1f.T))
    put("c2", _blkdiag(c2f.T, c2f.T))

    l1w, l1b = _np(params["l1w"]), _np(params["l1b"])
    l2w, l2b = _np(params["l2w"]), _np(params["l2b"])
    l3w, l3b = _np(params["l3w"]), _np(params["l3b"])
    w_eff = l3w @ l2w @ l1w                    # [1, 64]
    b_eff = float((l3w @ (l2w @ l1b + l2b) + l3b)[0])
    b_eff2 = b_eff + 0.5 * float(w_eff.sum())
    put("head", _blkdiag(w_eff.T, w_eff.T))
    put("id64", np.eye(64, dtype=np.float32))

    # biases ------------------------------------------------------------
    pe2 = _np(pe)[:, 0, :]  # [max_len, 64]
    per_core = []
    for c in range(N_CORES):
        bb = np.zeros((128, NB), np.float32)
        for lvl, (cin, cout) in enumerate(CH):
            p = tcn[lvl]
            b1 = np.concatenate([_np(p["b1"])] * 2)
            bs = np.concatenate([_np(p["b2"]) + _np(p["bd"])] * 2)
            bb[: 2 * cout, BIAS_COLS[f"b1_{lvl}"]] = b1
            bb[: 2 * cout, BIAS_COLS[f"bs_{lvl}"]] = bs
        pe_b0 = pe2[2 * c]
        pe_b1 = pe2[2 * c + 1]
        bv1 = np.concatenate([wv1 @ pe_b0 + _np(params["bv1"]),
                              wv1 @ pe_b1 + _np(params["bv1"])])
        bv2 = np.concatenate([wv2 @ pe_b0 + _np(params["bv2"]),
                              wv2 @ pe_b1 + _np(params["bv2"])])
        bb[:, BIAS_COLS["bv1"]] = bv1
        bb[:, BIAS_COLS["bv2"]] = bv2
        bb[:32, BIAS_COLS["b3f"]] = np.concatenate([b3f] * 2)
        bb[:32, BIAS_COLS["b1f"]] = np.concatenate([b1f] * 2)
        bb[:, BIAS_COLS["gconst"]] = np.concatenate([b2f + b4f] * 2)
        bb[:2, BIAS_COLS["beff"]] = b_eff2

        qkb_row = np.concatenate([
            wq @ pe_b0 + _np(params["bq"]), wq @ pe_b1 + _np(params["bq"]),
            wk @ pe_b0 + _np(params["bk"]), wk @ pe_b1 + _np(params["bk"]),
        ]).astype(np.float32)                                  # [256]
        qkb = np.ascontiguousarray(np.broadcast_to(qkb_row, (128, 256)))
        per_core.append((bb, qkb))

    return wblob, per_core


# ---------------------------------------------------------------------------
# device program
# ---------------------------------------------------------------------------

_PROG_CACHE = {}


def _build_program(T):
    from concourse import bacc, mybir, tile

    f32 = mybir.dt.float32
    slices, total = _wblob_layout()
    nblk = T // BLK
    ntb = T // TB

    nc = bacc.Bacc("TRN2", target_bir_lowering=False, debug=False,
                   num_devices=N_CORES)
    x_d = nc.dram_tensor("x", [2 * F_IN, T], f32, kind="ExternalInput")
    w_d = nc.dram_tensor("wts", [128, total], f32, kind="ExternalInput")
    b_d = nc.dram_tensor("biases", [128, NB], f32, kind="ExternalInput")
    qkb_d = nc.dram_tensor("qkb", [128, 256], f32, kind="ExternalInput")
    out_d = nc.dram_tensor("out", [2, T], f32, kind="ExternalOutput")

    from contextlib import ExitStack
    with tile.TileContext(nc) as tc, ExitStack() as ctx:
        consts = ctx.enter_context(tc.tile_pool(name="consts", bufs=1))
        fulls = ctx.enter_context(tc.tile_pool(name="fulls", bufs=1))
        work = ctx.enter_context(tc.tile_pool(name="work", bufs=2))
        small = ctx.enter_context(tc.tile_pool(name="small", bufs=2))
        ps = ctx.enter_context(tc.tile_pool(name="ps", bufs=4, space="PSUM"))
        ps_s = ctx.enter_context(tc.tile_pool(name="ps_s", bufs=1, space="PSUM"))

        wsb = consts.tile([128, total], f32, tag="wsb")
        nc.sync.dma_start(out=wsb[:], in_=w_d[:])
        bsb = consts.tile([128, NB], f32, tag="bsb")
        nc.sync.dma_start(out=bsb[:], in_=b_d[:])
        qkbias = consts.tile([128, 256], f32, tag="qkbias")
        nc.sync.dma_start(out=qkbias[:], in_=qkb_d[:])
        sT = consts.tile([128, 128], f32, tag="sT")

        def W(name):
            c0, m, k = slices[name]
            return wsb[0:k, c0:c0 + m]

        def BIAS(name, p):
            return bsb[0:p, BIAS_COLS[name]:BIAS_COLS[name] + 1]

        xs = fulls.tile([2 * F_IN, T], f32, tag="A")
        nc.sync.dma_start(out=xs[:], in_=x_d[:])

        # ---- TCN ----------------------------------------------------------
        relu = mybir.ActivationFunctionType.Relu
        sigmoid = mybir.ActivationFunctionType.Sigmoid
        expf = mybir.ActivationFunctionType.Exp

        tags_y = ["B", "A", "C"]   # conv1 outputs per level
        tags_h = ["C", "B", "A"]   # level outputs
        cur = xs
        for lvl, (cin, cout) in enumerate(CH):
            d = 2 ** lvl
            k2, m2 = 2 * cin, 2 * cout
            y = fulls.tile([m2, T], f32, tag=tags_y[lvl])
            for i in range(nblk):
                t0 = i * BLK
                pt = ps.tile([m2, BLK], f32, tag="mm")
                # taps: q=2 shift 0 (first, start), q=1 shift d, q=0 shift 2d
                nc.tensor.matmul(pt[:, :], W(f"w1_{lvl}_2"),
                                 cur[:, t0:t0 + BLK], start=True, stop=False)
                for q, s in ((1, d), (0, 2 * d)):
                    last = (q == 0)
                    if t0 - s >= 0:
                        nc.tensor.matmul(pt[:, :], W(f"w1_{lvl}_{q}"),
                                         cur[:, t0 - s:t0 - s + BLK],
                                         start=False, stop=last)
                    else:
                        nc.tensor.matmul(pt[:, s:BLK], W(f"w1_{lvl}_{q}"),
                                         cur[:, 0:BLK - s],
                                         start=False, stop=last)
                nc.scalar.activation(y[:, t0:t0 + BLK], pt[:, :], relu,
                                     bias=BIAS(f"b1_{lvl}", m2))
            h = fulls.tile([m2, T], f32, tag=tags_h[lvl])
            for i in range(nblk):
                t0 = i * BLK
                pt = ps.tile([m2, BLK], f32, tag="mm")
                nc.tensor.matmul(pt[:, :], W(f"w2_{lvl}_2"),
                                 y[:, t0:t0 + BLK], start=True, stop=False)
                for q, s in ((1, d), (0, 2 * d)):
                    if t0 - s >= 0:
                        nc.tensor.matmul(pt[:, :], W(f"w2_{lvl}_{q}"),
                                         y[:, t0 - s:t0 - s + BLK],
                                         start=False, stop=False)
                    else:
                        nc.tensor.matmul(pt[:, s:BLK], W(f"w2_{lvl}_{q}"),
                                         y[:, 0:BLK - s],
                                         start=False, stop=False)
                nc.tensor.matmul(pt[:, :], W(f"wd_{lvl}"),
                                 cur[:, t0:t0 + BLK], start=False, stop=True)
                nc.scalar.activation(h[:, t0:t0 + BLK], pt[:, :], relu,
                                     bias=BIAS(f"bs_{lvl}", m2))
            cur = h
        h = cur  # [128, T]

        # ---- V1 / V2 ------------------------------------------------------
        v1 = fulls.tile([128, T], f32, tag="D")
        v2 = fulls.tile([128, T], f32, tag="E")
        for i in range(nblk):
            t0 = i * BLK
            pv1 = ps.tile([128, BLK], f32, tag="mm")
            nc.tensor.matmul(pv1[:, :], W("v1"), h[:, t0:t0 + BLK],
                             start=True, stop=True)
            nc.vector.tensor_scalar_add(v1[:, t0:t0 + BLK], pv1[:, :],
                                        BIAS("bv1", 128))
            pv2 = ps.tile([128, BLK], f32, tag="mm")
            nc.tensor.matmul(pv2[:, :], W("v2"), h[:, t0:t0 + BLK],
                             start=True, stop=True)
            nc.vector.tensor_scalar_add(v2[:, t0:t0 + BLK], pv2[:, :],
                                        BIAS("bv2", 128))

        # ---- scores: s_b = sum_t qT kT ------------------------------------
        s0 = ps_s.tile([64, 64], f32, tag="s0")
        s1 = ps_s.tile([64, 64], f32, tag="s1")
        for tb in range(ntb):
            t0 = tb * TB
            pq = ps.tile([128, 256], f32, tag="mm")
            nc.tensor.matmul(pq[:, :], h[:, t0:t0 + TB], W("qk"),
                             start=True, stop=True)
            qk = work.tile([128, 256], f32, tag="qk")
            nc.vector.tensor_add(qk[:, :], pq[:, :], qkbias[:, :])
            nc.tensor.matmul(s0[:, :], qk[:, 0:64], qk[:, 128:192],
                             start=(tb == 0), stop=(tb == ntb - 1))
            nc.tensor.matmul(s1[:, :], qk[:, 64:128], qk[:, 192:256],
                             start=(tb == 0), stop=(tb == ntb - 1))

        # ---- softmax (x0.5) + transpose -> blkdiag sT ---------------------
        nc.vector.memset(sT[:], 0.0)
        for b, s_ps in ((0, s0), (1, s1)):
            mx = small.tile([64, 1], f32, tag="mx")
            nc.vector.tensor_reduce(mx[:], s_ps[:], axis=mybir.AxisListType.X,
                                    op=mybir.AluOpType.max)
            nmx = small.tile([64, 1], f32, tag="nmx")
            nc.vector.tensor_scalar_mul(nmx[:], mx[:], -1.0)
            e = work.tile([64, 64], f32, tag="e")
            rs = small.tile([64, 1], f32, tag="rs")
            nc.scalar.activation(e[:], s_ps[:], expf, bias=nmx[:],
                                 accum_out=rs[:])
            rinv = small.tile([64, 1], f32, tag="rinv")
            nc.vector.reciprocal(rinv[:], rs[:])
            nc.vector.tensor_scalar_mul(rinv[:], rinv[:], 0.5)
            nc.vector.tensor_scalar_mul(e[:], e[:], rinv[:])
            pT = ps.tile([64, 64], f32, tag="mm")
            nc.tensor.transpose(pT[:], e[:], W("id64"))
            if b == 0:
                nc.vector.tensor_copy(sT[0:64, 0:64], pT[:])
            else:
                tmpT = work.tile([64, 64], f32, tag="tmpT")
                nc.vector.tensor_copy(tmpT[:], pT[:])
                nc.sync.dma_start(out=sT[64:128, 64:128], in_=tmpT[:])

        # ---- o1' +/- o2' --------------------------------------------------
        cam = fulls.tile([128, T], f32, tag="B")
        dd = fulls.tile([128, T], f32, tag="C")
        for i in range(nblk):
            t0 = i * BLK
            po1 = ps.tile([128, BLK], f32, tag="mm")
            nc.tensor.matmul(po1[:, :], sT[:], v1[:, t0:t0 + BLK],
                             start=True, stop=True)
            po2 = ps.tile([128, BLK], f32, tag="mm")
            nc.tensor.matmul(po2[:, :], sT[:], v2[:, t0:t0 + BLK],
                             start=True, stop=True)
            o1s = work.tile([128, BLK], f32, tag="o1s")
            nc.vector.tensor_copy(o1s[:], po1[:, :])
            nc.vector.tensor_add(cam[:, t0:t0 + BLK], o1s[:], po2[:, :])
            nc.vector.tensor_sub(dd[:, t0:t0 + BLK], o1s[:], po2[:, :])

        # ---- CAM pooled branch -> gate bias -------------------------------
        mean = small.tile([128, 1], f32, tag="mean")
        nc.vector.tensor_reduce(mean[:], cam[:, :], axis=mybir.AxisListType.X,
                                op=mybir.AluOpType.add)
        nc.vector.tensor_scalar_mul(mean[:], mean[:], 1.0 / T)
        pr1 = ps.tile([32, 1], f32, tag="mm")
        nc.tensor.matmul(pr1[:, :], W("c1"), mean[:], start=True, stop=True)
        r1 = small.tile([32, 1], f32, tag="r1")
        nc.scalar.activation(r1[:], pr1[:], relu, bias=BIAS("b1f", 32))
        pa = ps.tile([128, 1], f32, tag="mm")
        nc.tensor.matmul(pa[:, :], W("c2"), r1[:], start=True, stop=True)
        gbias = small.tile([128, 1], f32, tag="gbias")
        nc.vector.tensor_add(gbias[:], pa[:, :], BIAS("gconst", 128))

        # ---- CAM conv branch + gate + head --------------------------------
        for i in range(nblk):
            t0 = i * BLK
            pc = ps.tile([32, BLK], f32, tag="mm")
            nc.tensor.matmul(pc[:, :], W("c3"), cam[:, t0:t0 + BLK],
                             start=True, stop=True)
            c16 = work.tile([32, BLK], f32, tag="c16")
            nc.scalar.activation(c16[:], pc[:, :], relu, bias=BIAS("b3f", 32))
            pg = ps.tile([128, BLK], f32, tag="mm")
            nc.tensor.matmul(pg[:, :], W("c4"), c16[:], start=True, stop=True)
            gate = work.tile([128, BLK], f32, tag="gate")
            nc.scalar.activation(gate[:], pg[:, :], sigmoid, bias=gbias[:])
            nc.vector.tensor_mul(gate[:], gate[:], dd[:, t0:t0 + BLK])
            ph = ps.tile([2, BLK], f32, tag="mm")
            nc.tensor.matmul(ph[:, :], W("head"), gate[:], start=True, stop=True)
            ob = work.tile([2, BLK], f32, tag="ob")
            nc.scalar.activation(ob[:], ph[:, :], sigmoid, bias=BIAS("beff", 2))
            nc.sync.dma_start(out=out_d[:, t0:t0 + BLK], in_=ob[:])

    nc.compile()
    return nc


def _get_program(T):
    if T not in _PROG_CACHE:
        _PROG_CACHE[T] = _build_program(T)
    return _PROG_CACHE[T]


# ---------------------------------------------------------------------------
# entry point
# ---------------------------------------------------------------------------

LAST_EXEC_NS = None
LAST_IN_MAPS = None


def _make_in_maps(x, params, pe):
    x = _np(x)
    B, T, F = x.shape
    assert B == B_FULL and F == F_IN
    wblob, per_core = _fold_params(params, pe)
    xt = np.ascontiguousarray(x.transpose(0, 2, 1))  # [B, 10, T]
    in_maps = []
    for c in range(N_CORES):
        bb, qkb = per_core[c]
        in_maps.append({
            "x": np.ascontiguousarray(
                xt[B_PER * c:B_PER * (c + 1)].reshape(B_PER * F_IN, T)),
            "wts": wblob,
            "biases": bb,
            "qkb": qkb,
        })
    return in_maps, T


def kernel(x, point_label=None, label=None, params=None, pe=None, **_ignored):
    global LAST_IN_MAPS
    from concourse.bass_utils import run_bass_kernel_spmd

    in_maps, T = _make_in_maps(x, params, pe)
    LAST_IN_MAPS = (in_maps, T)
    nc = _get_program(T)
    res = run_bass_kernel_spmd(nc, in_maps, core_ids=list(range(N_CORES)))
    out = np.zeros((B_FULL, 1, T), np.float32)
    for c in range(N_CORES):
        out[B_PER * c:B_PER * (c + 1), 0, :] = res.results[c]["out"]
    return out


# ---------------------------------------------------------------------------
# timing (no NTFF profiling available under this axon container, so measure
# marginal per-call latency with device-resident args, minus a nop baseline)
# ---------------------------------------------------------------------------

def _make_runner(nc):
    import jax
    from jax.sharding import Mesh, PartitionSpec
    from jax.experimental.shard_map import shard_map
    from concourse import mybir
    from concourse.bass2jax import (_bass_exec_p, partition_id_tensor,
                                    install_neuronx_cc_hook)
    install_neuronx_cc_hook()
    pname = nc.partition_id_tensor.name if nc.partition_id_tensor else None
    in_names, out_names, out_avals, zero_outs = [], [], [], []
    for alloc in nc.m.functions[0].allocations:
        if not isinstance(alloc, mybir.MemoryLocationSet):
            continue
        name = alloc.memorylocations[0].name
        if alloc.kind == "ExternalInput":
            if name != pname:
                in_names.append(name)
        elif alloc.kind == "ExternalOutput":
            shape = tuple(alloc.tensor_shape)
            dtype = mybir.dt.np(alloc.dtype)
            out_names.append(name)
            out_avals.append(jax.core.ShapedArray(shape, dtype))
            zero_outs.append(np.zeros(shape, dtype))
    n_params = len(in_names)
    all_in = in_names + out_names + ([pname] if pname else [])

    def _body(*args):
        operands = list(args)
        if pname:
            operands.append(partition_id_tensor())
        return tuple(_bass_exec_p.bind(
            *operands, out_avals=tuple(out_avals), in_names=tuple(all_in),
            out_names=tuple(out_names), lowering_input_output_aliases=(),
            sim_require_finite=True, sim_require_nnan=True, nc=nc))

    devices = jax.devices()[:N_CORES]
    mesh = Mesh(np.asarray(devices), ("core",))
    nin = n_params + len(out_names)
    fn = jax.jit(shard_map(_body, mesh=mesh,
                           in_specs=(PartitionSpec("core"),) * nin,
                           out_specs=(PartitionSpec("core"),) * len(out_names),
                           check_rep=False), keep_unused=True)
    return fn, in_names, zero_outs, mesh


def _time_program(nc, in_maps, iters):
    import jax, time
    from jax.sharding import NamedSharding, PartitionSpec
    fn, in_names, zero_outs, mesh = _make_runner(nc)
    sh = NamedSharding(mesh, PartitionSpec("core"))
    concat = [np.concatenate([m[n] for m in in_maps], axis=0) for n in in_names]
    concat += [np.concatenate([z] * N_CORES, axis=0) for z in zero_outs]
    dev = [jax.device_put(a, sh) for a in concat]
    out = fn(*dev)
    jax.block_until_ready(out)
    t0 = time.perf_counter()
    for _ in range(iters):
        out = fn(*dev)
    jax.block_until_ready(out)
    return (time.perf_counter() - t0) / iters, out


def _build_nop():
    from concourse import bacc, mybir, tile
    f32 = mybir.dt.float32
    nc = bacc.Bacc("TRN2", target_bir_lowering=False, debug=False,
                   num_devices=N_CORES)
    xd = nc.dram_tensor("x", [2, 64], f32, kind="ExternalInput")
    yd = nc.dram_tensor("y", [2, 64], f32, kind="ExternalOutput")
    with tile.TileContext(nc) as tc:
        with tc.tile_pool(name="p", bufs=1) as pool:
            t = pool.tile([2, 64], f32)
            nc.sync.dma_start(out=t[:], in_=xd[:])
            nc.sync.dma_start(out=yd[:], in_=t[:])
    nc.compile()
    return nc


def time_exec(iters=30):
    """Marginal per-call wall time of the kernel minus nop baseline, in ns."""
    global LAST_EXEC_NS
    assert LAST_IN_MAPS is not None, "call kernel() first"
    in_maps, T = LAST_IN_MAPS
    t_kernel, _ = _time_program(_get_program(T), in_maps, iters)
    nop = _build_nop()
    nop_maps = [{"x": np.zeros((2, 64), np.float32)} for _ in range(N_CORES)]
    t_nop, _ = _time_program(nop, nop_maps, iters)
    LAST_EXEC_NS = (t_kernel - t_nop) * 1e9
    return t_kernel * 1e9, t_nop * 1e9, LAST_EXEC_NS
